# revision 1
# baseline (speedup 1.0000x reference)
"""Distributed MHA kernel for Trainium2 (8 NeuronCores, SPMD).

Problem: b=2, s=2048, e=2048, 32 heads x 64 dim, rotary_dim=32, causal,
fp32 reference.  Sharding: core c = batch*4 + head_group, i.e. each core
handles one batch and 8 heads (tensor-parallel over heads, data-parallel
over batch).  Column-parallel Wqkv, row-parallel Wout; the 4 partial
outputs per batch are summed on the host (cheap: 4 x 16.8 MB).

Per-core math (all matmuls in bf16, accumulation fp32 in PSUM):
  qkvT = Wqkv_g^T x^T           (f on partitions; q,k get RoPE via DVE)
  v    = x Wv_g                 (s on partitions, +ones column for denom)
  sT[u,t] = kT^T qT             (scoresT; exp on ACT, no max-sub needed:
                                 logits are O(5), fp32 exp is safe)
  oT[65,t] += v_ext^T pT        (row 64 accumulates the softmax denom)
  aT = oT * bcast(1/den)        (denominator broadcast via K=1 matmul)
  y[t,e] = aT^T Wout_g          (partial; host sums groups + bias)
"""

import numpy as np

S = 2048
E = 2048
NET = 16          # e-tiles of 128
SCH = 512         # s-chunk
NCH = 4           # s-chunks


def _build_nc():
    import concourse.bacc as bacc
    import concourse.bass as bass  # noqa: F401
    import concourse.tile as tile
    from concourse import mybir

    f32 = mybir.dt.float32
    bf16 = mybir.dt.bfloat16
    AF = mybir.ActivationFunctionType

    nc = bacc.Bacc(None, target_bir_lowering=False)
    xT = nc.dram_tensor("xT", [E, S], bf16, kind="ExternalInput")
    wqkv = nc.dram_tensor("wqkv", [E, 1536], bf16, kind="ExternalInput")
    wout = nc.dram_tensor("wout", [512, E], bf16, kind="ExternalInput")
    bqk = nc.dram_tensor("bqk", [128, 8], f32, kind="ExternalInput")
    bv = nc.dram_tensor("bv", [128, 512], f32, kind="ExternalInput")
    crep = nc.dram_tensor("crep", [128, S], bf16, kind="ExternalInput")
    srep = nc.dram_tensor("srep", [128, S], bf16, kind="ExternalInput")
    masks = nc.dram_tensor("masks", [4, 128, SCH], bf16, kind="ExternalInput")
    y = nc.dram_tensor("y", [S, E], f32, kind="ExternalOutput")

    with tile.TileContext(nc) as tc:
        from contextlib import ExitStack

        with ExitStack() as ctx:
            consts = ctx.enter_context(tc.tile_pool(name="consts", bufs=1))
            qkp = ctx.enter_context(tc.tile_pool(name="qkp", bufs=1))
            qjp = ctx.enter_context(tc.tile_pool(name="qjp", bufs=2))
            vp = ctx.enter_context(tc.tile_pool(name="vp", bufs=1))
            atp = ctx.enter_context(tc.tile_pool(name="atp", bufs=1))
            xp = ctx.enter_context(tc.tile_pool(name="xp", bufs=2))
            ptp = ctx.enter_context(tc.tile_pool(name="ptp", bufs=3))
            rtp = ctx.enter_context(tc.tile_pool(name="rtp", bufs=2))
            dnp = ctx.enter_context(tc.tile_pool(name="dnp", bufs=1))
            rbp = ctx.enter_context(tc.tile_pool(name="rbp", bufs=2))
            yp_sb = ctx.enter_context(tc.tile_pool(name="yp_sb", bufs=2))
            ps_a = ctx.enter_context(
                tc.tile_pool(name="ps_a", bufs=2, space="PSUM"))
            ps_s = ctx.enter_context(
                tc.tile_pool(name="ps_s", bufs=2, space="PSUM"))
            ps_o = ctx.enter_context(
                tc.tile_pool(name="ps_o", bufs=1, space="PSUM"))
            ps_m = ctx.enter_context(
                tc.tile_pool(name="ps_m", bufs=2, space="PSUM"))

            # ---- resident constants
            w_sb = []
            for et in range(NET):
                t = consts.tile([128, 1536], bf16, tag=f"w{et}")
                nc.sync.dma_start(t, wqkv[et * 128:(et + 1) * 128, :])
                w_sb.append(t)
            wo_sb = []
            for pr in range(4):
                t = consts.tile([128, E], bf16, tag=f"wo{pr}")
                nc.sync.dma_start(t, wout[pr * 128:(pr + 1) * 128, :])
                wo_sb.append(t)
            crep_sb = consts.tile([128, S], bf16, tag="crep")
            nc.sync.dma_start(crep_sb, crep[:, :])
            srep_sb = consts.tile([128, S], bf16, tag="srep")
            nc.sync.dma_start(srep_sb, srep[:, :])
            bqk_sb = consts.tile([128, 8], f32, tag="bqk")
            nc.sync.dma_start(bqk_sb, bqk[:, :])
            bv_sb = consts.tile([128, 512], f32, tag="bv")
            nc.sync.dma_start(bv_sb, bv[:, :])
            mask_sb = []
            for k in range(4):
                t = consts.tile([128, SCH], bf16, tag=f"mask{k}")
                nc.sync.dma_start(t, masks[k, :, :])
                mask_sb.append(t)
            ones_sb = consts.tile([1, 64], bf16, tag="ones")
            nc.vector.memset(ones_sb, 1.0)

            qk_t = {}
            v_t = {}
            at_t = {}

            for tj in range(NCH):
                # ======== phase A: project chunk tj ========
                xs = []
                for et in range(NET):
                    t = xp.tile([128, SCH], bf16, tag=f"x{et}")
                    nc.sync.dma_start(
                        t, xT[et * 128:(et + 1) * 128,
                              tj * SCH:(tj + 1) * SCH])
                    xs.append(t)
                for ft in range(8):  # q: 0..3, k: 4..7
                    ps = ps_a.tile([128, 512], f32, tag="a")
                    for et in range(NET):
                        nc.tensor.matmul(
                            ps,
                            lhsT=w_sb[et][:, ft * 128:(ft + 1) * 128],
                            rhs=xs[et],
                            start=(et == 0), stop=(et == NET - 1))
                    if ft < 4:
                        qt = qjp.tile([128, SCH], bf16, tag=f"q{ft}")
                    else:
                        qt = qkp.tile([128, SCH], bf16, tag=f"k{ft}_{tj}")
                    nc.vector.tensor_scalar_add(qt, ps, bqk_sb[:, ft:ft + 1])
                    # RoPE (both 64-row head halves of this f-tile).
                    # The half-rotation swap is a +-16-partition shift;
                    # engine APs need 32-aligned partition starts, so do
                    # the swap with SBUF->SBUF DMAs instead.
                    tmp = rtp.tile([128, SCH], bf16, tag="rtmp")
                    cs = slice(tj * SCH, (tj + 1) * SCH)
                    for hh in (0, 64):
                        nc.sync.dma_start(
                            tmp[hh:hh + 16, :], qt[hh + 16:hh + 32, :])
                        nc.sync.dma_start(
                            tmp[hh + 16:hh + 32, :], qt[hh:hh + 16, :])
                    nc.vector.tensor_mul(qt, qt, crep_sb[:, cs])
                    for hh in (0, 64):
                        nc.vector.tensor_mul(
                            tmp[hh:hh + 32, :], tmp[hh:hh + 32, :],
                            srep_sb[hh:hh + 32, cs])
                        nc.vector.tensor_add(
                            qt[hh:hh + 32, :], qt[hh:hh + 32, :],
                            tmp[hh:hh + 32, :])
                    qk_t[(ft, tj)] = qt
                for us in range(4):
                    ut = tj * 4 + us
                    ps = ps_a.tile([128, 512], f32, tag="a")
                    for et in range(NET):
                        nc.tensor.matmul(
                            ps,
                            lhsT=xs[et][:, us * 128:(us + 1) * 128],
                            rhs=w_sb[et][:, 1024:1536],
                            start=(et == 0), stop=(et == NET - 1))
                    vt = vp.tile([128, 8, 65], bf16, tag=f"v{ut}")
                    nc.vector.tensor_add(
                        vt[:, :, 0:64],
                        ps.rearrange("p (h d) -> p h d", h=8),
                        bv_sb.rearrange("p (h d) -> p h d", h=8))
                    nc.vector.memset(vt[:, :, 64:65], 1.0)
                    v_t[ut] = vt

                # ======== phase B: attention rows t in chunk tj ========
                nu = 4 * tj + 4
                for pr in range(4):
                    oTs = []
                    for sub in range(2):
                        h = 2 * pr + sub
                        fq = h // 2
                        po = (h % 2) * 64
                        oT = ps_o.tile([65, 512], f32, tag=f"o{sub}")
                        for ut in range(nu):
                            jj, us = divmod(ut, 4)
                            st = ps_s.tile([128, 512], f32, tag="s")
                            nc.tensor.matmul(
                                st,
                                lhsT=qk_t[(4 + fq, jj)][po:po + 64,
                                                        us * 128:(us + 1) * 128],
                                rhs=qk_t[(fq, tj)][po:po + 64, :],
                                start=True, stop=True)
                            pt = ptp.tile([128, 512], bf16, tag="pt")
                            nc.scalar.activation(pt, st, AF.Exp, scale=0.125)
                            if ut >= 4 * tj:
                                nc.vector.tensor_mul(
                                    pt, pt, mask_sb[ut - 4 * tj])
                            nc.tensor.matmul(
                                oT, lhsT=v_t[ut][:, h, :], rhs=pt,
                                start=(ut == 0), stop=(ut == nu - 1))
                        oTs.append(oT)
                    # denominators -> reciprocal -> broadcast via K=1 matmul
                    rcbs = []
                    for sub in range(2):
                        dn = dnp.tile([1, 512], f32, tag=f"dn{sub}")
                        nc.vector.tensor_copy(dn, oTs[sub][64:65, :])
                        rc = dnp.tile([1, 512], f32, tag=f"rc{sub}")
                        nc.vector.reciprocal_approx_fast(out=rc, in_=dn)
                        rcb = dnp.tile([1, 512], bf16, tag=f"rcb{sub}")
                        nc.vector.tensor_copy(rcb, rc)
                        rcbs.append(rcb)
                    rb_ps = ps_m.tile([128, 512], f32, tag="m")
                    nc.tensor.matmul(rb_ps[0:64, :], lhsT=ones_sb,
                                     rhs=rcbs[0], start=True, stop=True)
                    nc.tensor.matmul(rb_ps[64:128, :], lhsT=ones_sb,
                                     rhs=rcbs[1], start=True, stop=True)
                    rb_sb = rbp.tile([128, 512], bf16, tag="rb")
                    nc.vector.tensor_copy(rb_sb, rb_ps)
                    at = atp.tile([128, 512], bf16, tag=f"at{pr}_{tj}")
                    nc.vector.tensor_mul(
                        at[0:64, :], oTs[0][0:64, :], rb_sb[0:64, :])
                    nc.vector.tensor_mul(
                        at[64:128, :], oTs[1][0:64, :], rb_sb[64:128, :])
                    at_t[(pr, tj)] = at

                # ======== phase C: output projection for chunk tj ========
                for ttl in range(4):
                    tt = tj * 4 + ttl
                    for ec in range(4):
                        yp = ps_m.tile([128, 512], f32, tag="m")
                        for pr in range(4):
                            nc.tensor.matmul(
                                yp,
                                lhsT=at_t[(pr, tj)][:, ttl * 128:(ttl + 1) * 128],
                                rhs=wo_sb[pr][:, ec * 512:(ec + 1) * 512],
                                start=(pr == 0), stop=(pr == 3))
                        ys = yp_sb.tile([128, 512], f32, tag="ys")
                        nc.vector.tensor_copy(ys, yp)
                        nc.sync.dma_start(
                            y[tt * 128:(tt + 1) * 128,
                              ec * 512:(ec + 1) * 512], ys)
    nc.compile()
    return nc


_CACHE = {}


def _host_consts():
    import ml_dtypes
    bf = ml_dtypes.bfloat16
    inv = 1.0 / (10000.0 ** (np.arange(0, 32, 2, dtype=np.float64) / 32.0))
    t = np.arange(S, dtype=np.float64)
    fr = np.outer(t, inv)                       # [s, 16]
    cos = np.cos(fr).astype(np.float32).T       # [16, s]
    sin = np.sin(fr).astype(np.float32).T
    crep = np.ones((128, S), np.float32)
    srep = np.zeros((128, S), np.float32)
    for blk in (0, 64):
        crep[blk:blk + 16] = cos
        crep[blk + 16:blk + 32] = cos
        srep[blk:blk + 16] = -sin
        srep[blk + 16:blk + 32] = sin
    m = np.zeros((4, 128, SCH), np.float32)
    ui = np.arange(128)[:, None]
    tcol = np.arange(SCH)[None, :]
    for k in range(4):
        m[k] = ((128 * k + ui) <= tcol).astype(np.float32)
    return crep.astype(bf), srep.astype(bf), m.astype(bf)


def kernel(**inputs):
    import ml_dtypes
    from concourse.bass_utils import run_bass_kernel_spmd

    x = np.asarray(inputs["x"], np.float32)
    Wqkv = np.asarray(inputs["Wqkv"], np.float32)
    bqkv = np.asarray(inputs["bqkv"], np.float32)
    Wout = np.asarray(inputs["Wout"], np.float32)
    bout = np.asarray(inputs["bout"], np.float32)

    if "nc" not in _CACHE:
        _CACHE["nc"] = _build_nc()
    nc = _CACHE["nc"]

    bf = ml_dtypes.bfloat16
    crep, srep, masks = _host_consts()
    in_maps = []
    for c in range(8):
        b, g = divmod(c, 4)
        gs = slice(g * 512, (g + 1) * 512)
        wq = Wqkv[:, 0:2048][:, gs]
        wk = Wqkv[:, 2048:4096][:, gs]
        wv = Wqkv[:, 4096:6144][:, gs]
        bq = bqkv[0:2048][gs]
        bk = bqkv[2048:4096][gs]
        bvv = bqkv[4096:6144][gs]
        in_maps.append(dict(
            xT=np.ascontiguousarray(x[b].T).astype(bf),
            wqkv=np.concatenate([wq, wk, wv], axis=1).astype(bf),
            wout=Wout[gs, :].astype(bf),
            bqk=np.concatenate([bq, bk]).reshape(8, 128).T.astype(
                np.float32).copy(),
            bv=np.broadcast_to(
                bvv.astype(np.float32), (128, 512)).copy(),
            crep=crep, srep=srep, masks=masks,
        ))
    kwargs = _CACHE.get("run_kwargs", {})
    res = run_bass_kernel_spmd(nc, in_maps, list(range(8)), **kwargs)
    _CACHE["last_results"] = res
    out = np.zeros((2, S, E), np.float32)
    for c in range(8):
        out[c // 4] += res.results[c]["y"]
    out += bout[None, None, :]
    return out



# revision 7
# speedup vs baseline: 1.1268x; 1.1268x over previous
"""Distributed MHA kernel for Trainium2 (8 NeuronCores, SPMD), v2.

Problem: b=2, s=2048, e=2048, 32 heads x 64 dim, rotary_dim=32, causal,
fp32 reference.  Sharding: core c = batch*4 + head_group, i.e. each core
handles one batch and 8 heads (tensor-parallel over heads, data-parallel
over batch).  Column-parallel Wqkv, row-parallel Wout; the 4 partial
outputs per batch are summed on the host (bf16 partials, 4 x 8.4 MB).

Per-core structure (all matmuls bf16, fp32 PSUM accumulation):
  A(tj): qkvT f-tiles for s-chunk tj; q/k bias folded into the ACT-engine
         PSUM->SBUF copy (bias AP), RoPE on DVE (3 tensor_tensor ops,
         half-rotation via SBUF-SBUF DMAs on the gpsimd SWDGE queue);
         v bias added via a K=1 ones x bias-row matmul.
  B(tj): per head-pair (f-tile) pr: scores for both heads as a row-tiled
         matmul pair (lhsT at partitions 0-63 / 64-127 -> concurrent
         32x32-subarray execution), one exp ACTIVATE over the [128,1024]
         pair tile, triangular [128,128] mask-mul on diagonal tiles only,
         pv matmuls with rhs narrowed to the un-masked column range;
         row 64 of the v-extended matmul accumulates the softmax denom.
  C(tj): output projection; per-(t,e) block 4 accumulating matmuls; the
         evacuation casts to bf16 and DMAs to a block-packed y.
Emission order A(0) B(0) A(1) C(0) B(1) A(2) C(1) B(2) A(3) B(3) C(2)
C(3) keeps the tensor engine dense (HAM-warm) through the tail.
"""

import numpy as np

S = 2048
E = 2048
NET = 16          # e-tiles of 128
SCH = 512         # s-chunk
NCH = 4           # s-chunks


def _build_nc():
    import concourse.bacc as bacc
    import concourse.bass as bass  # noqa: F401
    import concourse.tile as tile
    from concourse import mybir

    f32 = mybir.dt.float32
    bf16 = mybir.dt.bfloat16
    AF = mybir.ActivationFunctionType

    nc = bacc.Bacc(None, target_bir_lowering=False)
    # chunk-major xT so every x-tile DMA is a dense contiguous read
    xc = nc.dram_tensor("xc", [NCH, E, SCH], bf16, kind="ExternalInput")
    # f-tile-major q,k weights: wqk[o] = [E, 128] slab for f-tile o
    wqk = nc.dram_tensor("wqk", [8, E, 128], bf16, kind="ExternalInput")
    wv = nc.dram_tensor("wv", [E, 512], bf16, kind="ExternalInput")
    wout = nc.dram_tensor("wout", [512, E], bf16, kind="ExternalInput")
    bqk = nc.dram_tensor("bqk", [8, 128], bf16, kind="ExternalInput")
    bvr = nc.dram_tensor("bvr", [1, 512], bf16, kind="ExternalInput")
    crep = nc.dram_tensor("crep", [128, S], bf16, kind="ExternalInput")
    srep = nc.dram_tensor("srep", [128, S], bf16, kind="ExternalInput")
    tri = nc.dram_tensor("tri", [128, 128], bf16, kind="ExternalInput")
    # block-packed output: y[tt, ec] is one [128, 512] store
    y = nc.dram_tensor("y", [16, 4, 128, SCH], bf16, kind="ExternalOutput")

    with tile.TileContext(nc) as tc:
        from contextlib import ExitStack

        with ExitStack() as ctx:
            consts = ctx.enter_context(tc.tile_pool(name="consts", bufs=1))
            xp = ctx.enter_context(tc.tile_pool(name="xp", bufs=2))
            qjp = ctx.enter_context(tc.tile_pool(name="qjp", bufs=2))
            qkp = ctx.enter_context(tc.tile_pool(name="qkp", bufs=1))
            vp = ctx.enter_context(tc.tile_pool(name="vp", bufs=1))
            rtp = ctx.enter_context(tc.tile_pool(name="rtp", bufs=2))
            atp = ctx.enter_context(tc.tile_pool(name="atp", bufs=3))
            ptp = ctx.enter_context(tc.tile_pool(name="ptp", bufs=3))
            dnp = ctx.enter_context(tc.tile_pool(name="dnp", bufs=2))
            rbp = ctx.enter_context(tc.tile_pool(name="rbp", bufs=2))
            ysp = ctx.enter_context(tc.tile_pool(name="ysp", bufs=2))
            ps_a = ctx.enter_context(
                tc.tile_pool(name="ps_a", bufs=2, space="PSUM"))
            ps_s = ctx.enter_context(
                tc.tile_pool(name="ps_s", bufs=2, space="PSUM"))
            ps_o = ctx.enter_context(
                tc.tile_pool(name="ps_o", bufs=1, space="PSUM"))

            x_t = {}      # (et, tj) -> x tile
            q_t = {}      # (pr, tj) -> q f-tile (post-rope)
            k_t = {}      # (pr, tj) -> k f-tile (post-rope)
            v_t = {}      # ut -> v tile [128, 8, 65]
            at_t = {}     # (pr, tj) -> normalized attn out (transposed)

            def load_x(tj):
                for et in range(NET):
                    t = xp.tile([128, SCH], bf16, tag=f"x{et}")
                    nc.sync.dma_start(
                        t, xc[tj, et * 128:(et + 1) * 128, :])
                    x_t[(et, tj)] = t

            # ---- x chunk 0 first so the PE can start ASAP
            load_x(0)

            # ---- q/k weight slabs, f-tile-major: w_qk[o] = [128, 16, 128]
            w_qk = []
            for o in range(8):
                t = consts.tile([128, NET, 128], bf16, tag=f"wqk{o}")
                src = wqk[o].rearrange("(et p) c -> p et c", p=128)
                for g in range(4):
                    nc.sync.dma_start(
                        t[:, g * 4:(g + 1) * 4, :], src[:, g * 4:(g + 1) * 4, :])
                w_qk.append(t)
            # ---- v weights: w_v = [128, 16, 512]
            w_v = consts.tile([128, NET, 512], bf16, tag="wv")
            src = wv.rearrange("(et p) c -> p et c", p=128)
            for g in range(8):
                nc.sync.dma_start(
                    w_v[:, g * 2:(g + 1) * 2, :], src[:, g * 2:(g + 1) * 2, :])
            wo_sb = []
            for pr in range(4):
                t = consts.tile([128, E], bf16, tag=f"wo{pr}")
                nc.sync.dma_start(t, wout[pr * 128:(pr + 1) * 128, :])
                wo_sb.append(t)
            crep_sb = consts.tile([128, S], bf16, tag="crep")
            nc.sync.dma_start(crep_sb, crep[:, :])
            srep_sb = consts.tile([128, S], bf16, tag="srep")
            nc.sync.dma_start(srep_sb, srep[:, :])
            bqk_sb = []
            for o in range(8):
                t = consts.tile([1, 128], bf16, tag=f"bqk{o}")
                nc.sync.dma_start(t, bqk[o:o + 1, :])
                bqk_sb.append(t)
            bv_sb = consts.tile([1, 512], bf16, tag="bv")
            nc.sync.dma_start(bv_sb, bvr[:, :])
            tri_sb = consts.tile([128, 128], bf16, tag="tri")
            nc.sync.dma_start(tri_sb, tri[:, :])
            ones = consts.tile([1, 512], bf16, tag="ones")
            nc.vector.memset(ones, 1.0)

            def phase_a(tj):
                cs = slice(tj * SCH, (tj + 1) * SCH)
                for o in range(12):
                    ps = ps_a.tile([128, 512], f32, tag="a")
                    if o < 8:
                        # q (o 0-3) / k (o 4-7) f-tile: w^T x
                        for et in range(NET):
                            nc.tensor.matmul(
                                ps, lhsT=w_qk[o][:, et, :],
                                rhs=x_t[(et, tj)],
                                start=(et == 0), stop=False)
                        # + bias(128) x ones row: per-partition bias in PE
                        nc.tensor.matmul(
                            ps, lhsT=bqk_sb[o], rhs=ones,
                            start=False, stop=True)
                        pr = o if o < 4 else o - 4
                        if o < 4:
                            qt = qjp.tile([128, SCH], bf16, tag=f"q{pr}")
                            q_t[(pr, tj)] = qt
                        else:
                            qt = qkp.tile([128, SCH], bf16, tag=f"k{pr}_{tj}")
                            k_t[(pr, tj)] = qt
                        nc.scalar.activation(qt, ps, AF.Copy)
                        # RoPE: tmp = within-32-block 16-row swap of qt
                        # (rows 32-63/96-127 copied straight; srep is 0
                        # there).  SBUF-SBUF DMAs ride the gpsimd SWDGE
                        # queue to keep the sync HWDGE queue for loads.
                        tmp = rtp.tile([128, SCH], bf16, tag="rtmp")
                        for hh in (0, 64):
                            nc.gpsimd.dma_start(
                                tmp[hh:hh + 16, :], qt[hh + 16:hh + 32, :])
                            nc.gpsimd.dma_start(
                                tmp[hh + 16:hh + 32, :], qt[hh:hh + 16, :])
                            nc.gpsimd.dma_start(
                                tmp[hh + 32:hh + 64, :], qt[hh + 32:hh + 64, :])
                        nc.vector.tensor_mul(qt, qt, crep_sb[:, cs])
                        nc.vector.tensor_mul(tmp, tmp, srep_sb[:, cs])
                        nc.vector.tensor_add(qt, qt, tmp)
                    else:
                        us = o - 8
                        ut = tj * 4 + us
                        for et in range(NET):
                            nc.tensor.matmul(
                                ps,
                                lhsT=x_t[(et, tj)][:, us * 128:(us + 1) * 128],
                                rhs=w_v[:, et, :],
                                start=(et == 0), stop=False)
                        # + ones(128) x bv row: broadcast bias add in PE
                        nc.tensor.matmul(
                            ps, lhsT=ones[:, 0:128], rhs=bv_sb,
                            start=False, stop=True)
                        vt = vp.tile([128, 8, 65], bf16, tag=f"v{ut}")
                        nc.scalar.activation(
                            vt[:, :, 0:64],
                            ps.rearrange("p (h d) -> p h d", h=8), AF.Copy)
                        nc.vector.memset(vt[:, :, 64:65], 1.0)
                        v_t[ut] = vt

            def phase_b(tj):
                nu = 4 * tj + 4
                for pr in range(4):
                    h0, h1 = 2 * pr, 2 * pr + 1
                    oTa = ps_o.tile([65, 512], f32, tag="o0")
                    oTb = ps_o.tile([65, 512], f32, tag="o1")
                    qt = q_t[(pr, tj)]
                    for ut in range(nu):
                        jj, us = divmod(ut, 4)
                        kk = ut - 4 * tj  # >=0: diagonal tile index
                        kt = k_t[(pr, jj)]
                        pp = ps_s.tile([128, 1024], f32, tag="s")
                        # scores for both heads as a concurrent row-tiled
                        # pair (lhsT partitions 0-63 / 64-127)
                        nc.tensor.matmul(
                            pp[:, 0:512],
                            lhsT=kt[0:64, us * 128:(us + 1) * 128],
                            rhs=qt[0:64, :], start=True, stop=True)
                        nc.tensor.matmul(
                            pp[:, 512:1024],
                            lhsT=kt[64:128, us * 128:(us + 1) * 128],
                            rhs=qt[64:128, :], start=True, stop=True)
                        pt = ptp.tile([128, 1024], bf16, tag="pt")
                        nc.scalar.activation(pt, pp, AF.Exp, scale=0.125)
                        off = 0
                        if kk >= 0:
                            off = 128 * kk
                            nc.vector.tensor_mul(
                                pt[:, off:off + 128],
                                pt[:, off:off + 128], tri_sb)
                            nc.vector.tensor_mul(
                                pt[:, 512 + off:512 + off + 128],
                                pt[:, 512 + off:512 + off + 128], tri_sb)
                        nc.tensor.matmul(
                            oTa[:, off:512], lhsT=v_t[ut][:, h0, :],
                            rhs=pt[:, off:512],
                            start=(ut == 0), stop=(ut == nu - 1))
                        nc.tensor.matmul(
                            oTb[:, off:512], lhsT=v_t[ut][:, h1, :],
                            rhs=pt[:, 512 + off:1024],
                            start=(ut == 0), stop=(ut == nu - 1))
                    # denominators -> reciprocal -> broadcast via K=1 matmul
                    # (copy denom rows to SBUF first: the custom-DVE recip
                    # uop chain is not reliable with a PSUM source)
                    dna = dnp.tile([1, 512], f32, tag="dn0")
                    nc.vector.tensor_copy(dna, oTa[64:65, :])
                    dnb = dnp.tile([1, 512], f32, tag="dn1")
                    nc.vector.tensor_copy(dnb, oTb[64:65, :])
                    rca = dnp.tile([1, 512], f32, tag="rc0")
                    nc.vector.reciprocal_approx_fast(out=rca, in_=dna)
                    rcb = dnp.tile([1, 512], f32, tag="rc1")
                    nc.vector.reciprocal_approx_fast(out=rcb, in_=dnb)
                    rba = dnp.tile([1, 512], bf16, tag="rcb0")
                    nc.vector.tensor_copy(rba, rca)
                    rbb = dnp.tile([1, 512], bf16, tag="rcb1")
                    nc.vector.tensor_copy(rbb, rcb)
                    rb_ps = ps_s.tile([128, 512], f32, tag="s")
                    nc.tensor.matmul(rb_ps[0:64, :], lhsT=ones[:, 0:64],
                                     rhs=rba, start=True, stop=True)
                    nc.tensor.matmul(rb_ps[64:128, :], lhsT=ones[:, 0:64],
                                     rhs=rbb, start=True, stop=True)
                    rb_sb = rbp.tile([128, 512], bf16, tag="rb")
                    nc.vector.tensor_copy(rb_sb, rb_ps)
                    at = atp.tile([128, 512], bf16, tag=f"at{pr}")
                    nc.vector.tensor_mul(
                        at[0:64, :], oTa[0:64, :], rb_sb[0:64, :])
                    nc.vector.tensor_mul(
                        at[64:128, :], oTb[0:64, :], rb_sb[64:128, :])
                    at_t[(pr, tj)] = at

            def phase_c(tj):
                for ttl in range(4):
                    tt = tj * 4 + ttl
                    for ec in range(4):
                        yp = ps_s.tile([128, 512], f32, tag="s")
                        for pr in range(4):
                            nc.tensor.matmul(
                                yp,
                                lhsT=at_t[(pr, tj)][:, ttl * 128:(ttl + 1) * 128],
                                rhs=wo_sb[pr][:, ec * 512:(ec + 1) * 512],
                                start=(pr == 0), stop=(pr == 3))
                        ys = ysp.tile([128, 512], bf16, tag="ys")
                        nc.vector.tensor_copy(ys, yp)
                        nc.sync.dma_start(y[tt, ec, :, :], ys)

            # emission order == scheduling priority: keep B (which feeds
            # the scalar engine) hot, A one chunk ahead as PE filler, and
            # defer C(2)/C(3) to give the PE work under the tail's exps.
            phase_a(0)
            load_x(1)
            phase_b(0)
            phase_a(1)
            phase_c(0)
            load_x(2)
            phase_b(1)
            phase_a(2)
            phase_c(1)
            load_x(3)
            phase_b(2)
            phase_a(3)
            phase_b(3)
            phase_c(2)
            phase_c(3)
    nc.compile()
    return nc


_CACHE = {}


def _host_consts():
    import ml_dtypes
    bf = ml_dtypes.bfloat16
    inv = 1.0 / (10000.0 ** (np.arange(0, 32, 2, dtype=np.float64) / 32.0))
    t = np.arange(S, dtype=np.float64)
    fr = np.outer(t, inv)                       # [s, 16]
    cos = np.cos(fr).astype(np.float32).T       # [16, s]
    sin = np.sin(fr).astype(np.float32).T
    crep = np.ones((128, S), np.float32)
    srep = np.zeros((128, S), np.float32)
    for blk in (0, 64):
        crep[blk:blk + 16] = cos
        crep[blk + 16:blk + 32] = cos
        srep[blk:blk + 16] = -sin
        srep[blk + 16:blk + 32] = sin
    ui = np.arange(128)[:, None]
    cc = np.arange(128)[None, :]
    tri = (ui <= cc).astype(np.float32)         # keep[u, c]
    return crep.astype(bf), srep.astype(bf), tri.astype(bf)


def kernel(**inputs):
    import ml_dtypes
    from concourse.bass_utils import run_bass_kernel_spmd

    x = np.asarray(inputs["x"], np.float32)
    Wqkv = np.asarray(inputs["Wqkv"], np.float32)
    bqkv = np.asarray(inputs["bqkv"], np.float32)
    Wout = np.asarray(inputs["Wout"], np.float32)
    bout = np.asarray(inputs["bout"], np.float32)

    if "nc" not in _CACHE:
        _CACHE["nc"] = _build_nc()
    nc = _CACHE["nc"]

    bf = ml_dtypes.bfloat16
    crep, srep, tri = _host_consts()
    in_maps = []
    for c in range(8):
        b, g = divmod(c, 4)
        gs = slice(g * 512, (g + 1) * 512)
        wq = Wqkv[:, 0:2048][:, gs]
        wk = Wqkv[:, 2048:4096][:, gs]
        wvv = Wqkv[:, 4096:6144][:, gs]
        bq = bqkv[0:2048][gs]
        bk = bqkv[2048:4096][gs]
        bvv = bqkv[4096:6144][gs]
        xT = np.ascontiguousarray(x[b].T)                  # [E, S]
        xc = np.ascontiguousarray(
            xT.reshape(E, NCH, SCH).transpose(1, 0, 2))    # [NCH, E, SCH]
        wqk = np.ascontiguousarray(
            np.concatenate([wq, wk], axis=1)               # [E, 1024]
            .reshape(E, 8, 128).transpose(1, 0, 2))        # [8, E, 128]
        in_maps.append(dict(
            xc=xc.astype(bf),
            wqk=wqk.astype(bf),
            wv=wvv.astype(bf),
            wout=Wout[gs, :].astype(bf),
            bqk=np.concatenate([bq, bk]).reshape(8, 128).astype(bf),
            bvr=bvv.reshape(1, 512).astype(bf),
            crep=crep, srep=srep, tri=tri,
        ))
    kwargs = _CACHE.get("run_kwargs", {})
    res = run_bass_kernel_spmd(nc, in_maps, list(range(8)), **kwargs)
    _CACHE["last_results"] = res
    out = np.zeros((2, S, E), np.float32)
    for c in range(8):
        yb = np.asarray(res.results[c]["y"], np.float32)   # [16,4,128,512]
        out[c // 4] += yb.transpose(0, 2, 1, 3).reshape(S, E)
    out += bout[None, None, :]
    return out


# revision 8
# speedup vs baseline: 1.2538x; 1.1126x over previous
"""Distributed MHA kernel for Trainium2 (8 NeuronCores, SPMD), v3.

Problem: b=2, s=2048, e=2048, 32 heads x 64 dim, rotary_dim=32, causal,
fp32 reference.  Sharding: core c = batch*4 + head_group, i.e. each core
handles one batch and 8 heads (tensor-parallel over heads, data-parallel
over batch).  Column-parallel Wqkv, row-parallel Wout; the 4 partial
outputs per batch are summed on the host (bf16 partials, 4 x 8.4 MB).

Per-core structure (all matmuls bf16, fp32 PSUM accumulation):
  A(tj): qkvT f-tiles for s-chunk tj; bias folded into the DVE PSUM->SBUF
         evacuation; RoPE on DVE (3 tensor_tensor ops; half-rotation via
         SBUF-SBUF DMAs on the gpsimd SWDGE queue).
  B(tj): per head-pair (f-tile) pr: scores for both heads as a row-tiled
         matmul pair (lhsT at partitions 0-63 / 64-127 -> concurrent
         32x32-subarray execution), one exp ACTIVATE over the [128,1024]
         pair tile, triangular [128,128] mask-mul on diagonal tiles only,
         pv matmuls with rhs narrowed to the un-masked column range;
         row 64 of the v-extended matmul accumulates the softmax denom.
  C(tj): output projection; 4 accumulating matmuls per (t,e) block; bf16
         row-block stores (one DMA per 128-row block).
Emission order A0 B0 A1 C0 B1 A2 B2 A3 B3 C1 C2 C3: A runs one chunk
ahead of B as tensor-engine filler, and the deferred C phases keep the
PE dense (HAM-warm) while the scalar engine works through the last
chunk's exps.  Input DMAs are batched into few large descriptors and
split across both HWDGE queues (sync + scalar) so issue serialization
does not gate the first chunk.
"""

import numpy as np

S = 2048
E = 2048
NET = 16          # e-tiles of 128
SCH = 512         # s-chunk
NCH = 4           # s-chunks


def _build_nc():
    import concourse.bacc as bacc
    import concourse.bass as bass  # noqa: F401
    import concourse.tile as tile
    from concourse import mybir

    f32 = mybir.dt.float32
    bf16 = mybir.dt.bfloat16
    AF = mybir.ActivationFunctionType

    nc = bacc.Bacc(None, target_bir_lowering=False)
    # chunk-major xT so every x load is a dense contiguous read
    xc = nc.dram_tensor("xc", [NCH, E, SCH], bf16, kind="ExternalInput")
    # f-tile-major q,k weights: wqk[o] = [E, 128] slab for f-tile o
    wqk = nc.dram_tensor("wqk", [8, E, 128], bf16, kind="ExternalInput")
    wv = nc.dram_tensor("wv", [E, 512], bf16, kind="ExternalInput")
    wout = nc.dram_tensor("wout", [512, E], bf16, kind="ExternalInput")
    bqk = nc.dram_tensor("bqk", [128, 8], f32, kind="ExternalInput")
    bvb = nc.dram_tensor("bvb", [128, 512], bf16, kind="ExternalInput")
    crep = nc.dram_tensor("crep", [128, S], bf16, kind="ExternalInput")
    srep = nc.dram_tensor("srep", [128, S], bf16, kind="ExternalInput")
    tri = nc.dram_tensor("tri", [128, 128], bf16, kind="ExternalInput")
    # block-packed output: y[tt] is one [128, E] store
    y = nc.dram_tensor("y", [16, 128, E], bf16, kind="ExternalOutput")

    with tile.TileContext(nc) as tc:
        from contextlib import ExitStack

        with ExitStack() as ctx:
            consts = ctx.enter_context(tc.tile_pool(name="consts", bufs=1))
            xp = ctx.enter_context(tc.tile_pool(name="xp", bufs=2))
            qjp = ctx.enter_context(tc.tile_pool(name="qjp", bufs=2))
            qkp = ctx.enter_context(tc.tile_pool(name="qkp", bufs=1))
            vp = ctx.enter_context(tc.tile_pool(name="vp", bufs=1))
            rtp = ctx.enter_context(tc.tile_pool(name="rtp", bufs=2))
            atp = ctx.enter_context(tc.tile_pool(name="atp", bufs=3))
            ptp = ctx.enter_context(tc.tile_pool(name="ptp", bufs=3))
            dnp = ctx.enter_context(tc.tile_pool(name="dnp", bufs=2))
            rbp = ctx.enter_context(tc.tile_pool(name="rbp", bufs=2))
            ysp = ctx.enter_context(tc.tile_pool(name="ysp", bufs=2))
            ps_a = ctx.enter_context(
                tc.tile_pool(name="ps_a", bufs=2, space="PSUM"))
            ps_s = ctx.enter_context(
                tc.tile_pool(name="ps_s", bufs=2, space="PSUM"))
            ps_o = ctx.enter_context(
                tc.tile_pool(name="ps_o", bufs=1, space="PSUM"))

            x_t = {}      # tj -> x tile [128, 16, 512]
            q_t = {}      # (pr, tj) -> q f-tile (post-rope)
            k_t = {}      # (pr, tj) -> k f-tile (post-rope)
            v_t = {}      # ut -> v tile [128, 8, 65]
            at_t = {}     # (pr, tj) -> normalized attn out (transposed)

            def load_x(tj):
                t = xp.tile([128, NET, SCH], bf16, tag="x")
                src = xc[tj].rearrange("(et p) c -> p et c", p=128)
                for g in range(4):
                    eng = nc.sync if g % 2 == 0 else nc.scalar
                    eng.dma_start(
                        t[:, g * 4:(g + 1) * 4, :], src[:, g * 4:(g + 1) * 4, :])
                x_t[tj] = t

            # ---- x chunk 0 + early consts first so the PE starts ASAP;
            # loads alternate between the two HWDGE queues (sync/scalar).
            load_x(0)
            w_qk = []
            for o in range(8):
                t = consts.tile([128, NET, 128], bf16, tag=f"wqk{o}")
                src = wqk[o].rearrange("(et p) c -> p et c", p=128)
                eng = nc.sync if o % 2 == 0 else nc.scalar
                eng.dma_start(t, src)
                w_qk.append(t)
                if o == 0:
                    crep_sb = consts.tile([128, S], bf16, tag="crep")
                    nc.scalar.dma_start(crep_sb, crep[:, :])
                    srep_sb = consts.tile([128, S], bf16, tag="srep")
                    nc.sync.dma_start(srep_sb, srep[:, :])
                    bqk_sb = consts.tile([128, 8], f32, tag="bqk")
                    nc.scalar.dma_start(bqk_sb, bqk[:, :])
            # ---- v weights: w_v = [128, 16, 512]
            w_v = consts.tile([128, NET, 512], bf16, tag="wv")
            src = wv.rearrange("(et p) c -> p et c", p=128)
            for g in range(2):
                eng = nc.sync if g % 2 == 0 else nc.scalar
                eng.dma_start(
                    w_v[:, g * 8:(g + 1) * 8, :], src[:, g * 8:(g + 1) * 8, :])
            bv_sb = consts.tile([128, 512], bf16, tag="bv")
            nc.sync.dma_start(bv_sb, bvb[:, :])
            tri_sb = consts.tile([128, 128], bf16, tag="tri")
            nc.scalar.dma_start(tri_sb, tri[:, :])
            wo_sb = consts.tile([128, 4, E], bf16, tag="wo")
            src = wout.rearrange("(pr p) c -> p pr c", p=128)
            for g in range(2):
                eng = nc.sync if g % 2 == 0 else nc.scalar
                eng.dma_start(
                    wo_sb[:, g * 2:(g + 1) * 2, :], src[:, g * 2:(g + 1) * 2, :])
            ones = consts.tile([1, 512], bf16, tag="ones")
            nc.vector.memset(ones, 1.0)

            def phase_a(tj):
                cs = slice(tj * SCH, (tj + 1) * SCH)
                xs = x_t[tj]
                for o in range(12):
                    ps = ps_a.tile([128, 512], f32, tag="a")
                    if o < 8:
                        # q (o 0-3) / k (o 4-7) f-tile: w^T x
                        for et in range(NET):
                            nc.tensor.matmul(
                                ps, lhsT=w_qk[o][:, et, :],
                                rhs=xs[:, et, :],
                                start=(et == 0), stop=(et == NET - 1))
                        pr = o if o < 4 else o - 4
                        if o < 4:
                            qt = qjp.tile([128, SCH], bf16, tag=f"q{pr}")
                            q_t[(pr, tj)] = qt
                        else:
                            qt = qkp.tile([128, SCH], bf16, tag=f"k{pr}_{tj}")
                            k_t[(pr, tj)] = qt
                        # PSUM->SBUF evacuation with per-partition bias
                        nc.vector.tensor_scalar_add(
                            qt, ps, bqk_sb[:, o:o + 1])
                        # RoPE: tmp = within-32-block 16-row swap of qt
                        # (rows 32-63/96-127 copied straight; srep is 0
                        # there).  SBUF-SBUF DMAs ride the gpsimd SWDGE
                        # queue to keep the HWDGE queues for loads.
                        tmp = rtp.tile([128, SCH], bf16, tag="rtmp")
                        for hh in (0, 64):
                            nc.gpsimd.dma_start(
                                tmp[hh:hh + 16, :], qt[hh + 16:hh + 32, :])
                            nc.gpsimd.dma_start(
                                tmp[hh + 16:hh + 32, :], qt[hh:hh + 16, :])
                            nc.gpsimd.dma_start(
                                tmp[hh + 32:hh + 64, :], qt[hh + 32:hh + 64, :])
                        nc.vector.tensor_mul(qt, qt, crep_sb[:, cs])
                        nc.vector.tensor_mul(tmp, tmp, srep_sb[:, cs])
                        nc.vector.tensor_add(qt, qt, tmp)
                    else:
                        us = o - 8
                        ut = tj * 4 + us
                        for et in range(NET):
                            nc.tensor.matmul(
                                ps,
                                lhsT=xs[:, et, us * 128:(us + 1) * 128],
                                rhs=w_v[:, et, :],
                                start=(et == 0), stop=(et == NET - 1))
                        vt = vp.tile([128, 8, 65], bf16, tag=f"v{ut}")
                        nc.vector.tensor_add(
                            vt[:, :, 0:64],
                            ps.rearrange("p (h d) -> p h d", h=8),
                            bv_sb.rearrange("p (h d) -> p h d", h=8))
                        nc.vector.memset(vt[:, :, 64:65], 1.0)
                        v_t[ut] = vt

            def phase_b(tj):
                nu = 4 * tj + 4
                for pr in range(4):
                    h0, h1 = 2 * pr, 2 * pr + 1
                    oTa = ps_o.tile([65, 512], f32, tag="o0")
                    oTb = ps_o.tile([65, 512], f32, tag="o1")
                    qt = q_t[(pr, tj)]
                    for ut in range(nu):
                        jj, us = divmod(ut, 4)
                        kk = ut - 4 * tj  # >=0: diagonal tile index
                        kt = k_t[(pr, jj)]
                        pp = ps_s.tile([128, 1024], f32, tag="s")
                        # scores for both heads as a concurrent row-tiled
                        # pair (lhsT partitions 0-63 / 64-127)
                        nc.tensor.matmul(
                            pp[:, 0:512],
                            lhsT=kt[0:64, us * 128:(us + 1) * 128],
                            rhs=qt[0:64, :], start=True, stop=True)
                        nc.tensor.matmul(
                            pp[:, 512:1024],
                            lhsT=kt[64:128, us * 128:(us + 1) * 128],
                            rhs=qt[64:128, :], start=True, stop=True)
                        pt = ptp.tile([128, 1024], bf16, tag="pt")
                        nc.scalar.activation(pt, pp, AF.Exp, scale=0.125)
                        off = 0
                        if kk >= 0:
                            off = 128 * kk
                            nc.vector.tensor_mul(
                                pt[:, off:off + 128],
                                pt[:, off:off + 128], tri_sb)
                            nc.vector.tensor_mul(
                                pt[:, 512 + off:512 + off + 128],
                                pt[:, 512 + off:512 + off + 128], tri_sb)
                        nc.tensor.matmul(
                            oTa[:, off:512], lhsT=v_t[ut][:, h0, :],
                            rhs=pt[:, off:512],
                            start=(ut == 0), stop=(ut == nu - 1))
                        nc.tensor.matmul(
                            oTb[:, off:512], lhsT=v_t[ut][:, h1, :],
                            rhs=pt[:, 512 + off:1024],
                            start=(ut == 0), stop=(ut == nu - 1))
                    # denominators -> reciprocal -> broadcast via K=1 matmul
                    # (denominator rows go PSUM->SBUF first: the custom-DVE
                    # recip uop chain is not reliable with a PSUM source)
                    dna = dnp.tile([1, 512], f32, tag="dn0")
                    nc.vector.tensor_copy(dna, oTa[64:65, :])
                    dnb = dnp.tile([1, 512], f32, tag="dn1")
                    nc.vector.tensor_copy(dnb, oTb[64:65, :])
                    rca = dnp.tile([1, 512], f32, tag="rc0")
                    nc.vector.reciprocal_approx_fast(out=rca, in_=dna)
                    rcb = dnp.tile([1, 512], f32, tag="rc1")
                    nc.vector.reciprocal_approx_fast(out=rcb, in_=dnb)
                    rba = dnp.tile([1, 512], bf16, tag="rcb0")
                    nc.vector.tensor_copy(rba, rca)
                    rbb = dnp.tile([1, 512], bf16, tag="rcb1")
                    nc.vector.tensor_copy(rbb, rcb)
                    rb_ps = ps_s.tile([128, 512], f32, tag="s")
                    nc.tensor.matmul(rb_ps[0:64, :], lhsT=ones[:, 0:64],
                                     rhs=rba, start=True, stop=True)
                    nc.tensor.matmul(rb_ps[64:128, :], lhsT=ones[:, 0:64],
                                     rhs=rbb, start=True, stop=True)
                    rb_sb = rbp.tile([128, 512], bf16, tag="rb")
                    nc.vector.tensor_copy(rb_sb, rb_ps)
                    at = atp.tile([128, 512], bf16, tag=f"at{pr}")
                    nc.vector.tensor_mul(
                        at[0:64, :], oTa[0:64, :], rb_sb[0:64, :])
                    nc.vector.tensor_mul(
                        at[64:128, :], oTb[0:64, :], rb_sb[64:128, :])
                    at_t[(pr, tj)] = at

            def phase_c(tj):
                for ttl in range(4):
                    tt = tj * 4 + ttl
                    ys = ysp.tile([128, E], bf16, tag="ys")
                    for ec in range(4):
                        yp = ps_s.tile([128, 512], f32, tag="s")
                        for pr in range(4):
                            nc.tensor.matmul(
                                yp,
                                lhsT=at_t[(pr, tj)][:, ttl * 128:(ttl + 1) * 128],
                                rhs=wo_sb[:, pr, ec * 512:(ec + 1) * 512],
                                start=(pr == 0), stop=(pr == 3))
                        nc.vector.tensor_copy(
                            ys[:, ec * 512:(ec + 1) * 512], yp)
                    nc.sync.dma_start(y[tt, :, :], ys)

            # emission order == scheduling priority: B (which feeds the
            # scalar engine) hot, A one chunk ahead as PE filler, C(1..3)
            # deferred to keep the PE dense under the tail's exps.
            phase_a(0)
            load_x(1)
            phase_b(0)
            phase_a(1)
            phase_c(0)
            load_x(2)
            phase_b(1)
            phase_a(2)
            load_x(3)
            phase_b(2)
            phase_a(3)
            phase_b(3)
            phase_c(1)
            phase_c(2)
            phase_c(3)
    nc.compile()
    return nc


_CACHE = {}


def _host_consts():
    import ml_dtypes
    bf = ml_dtypes.bfloat16
    inv = 1.0 / (10000.0 ** (np.arange(0, 32, 2, dtype=np.float64) / 32.0))
    t = np.arange(S, dtype=np.float64)
    fr = np.outer(t, inv)                       # [s, 16]
    cos = np.cos(fr).astype(np.float32).T       # [16, s]
    sin = np.sin(fr).astype(np.float32).T
    crep = np.ones((128, S), np.float32)
    srep = np.zeros((128, S), np.float32)
    for blk in (0, 64):
        crep[blk:blk + 16] = cos
        crep[blk + 16:blk + 32] = cos
        srep[blk:blk + 16] = -sin
        srep[blk + 16:blk + 32] = sin
    ui = np.arange(128)[:, None]
    cc = np.arange(128)[None, :]
    tri = (ui <= cc).astype(np.float32)         # keep[u, c]
    return crep.astype(bf), srep.astype(bf), tri.astype(bf)


def kernel(**inputs):
    import ml_dtypes
    from concourse.bass_utils import run_bass_kernel_spmd

    x = np.asarray(inputs["x"], np.float32)
    Wqkv = np.asarray(inputs["Wqkv"], np.float32)
    bqkv = np.asarray(inputs["bqkv"], np.float32)
    Wout = np.asarray(inputs["Wout"], np.float32)
    bout = np.asarray(inputs["bout"], np.float32)

    if "nc" not in _CACHE:
        _CACHE["nc"] = _build_nc()
    nc = _CACHE["nc"]

    bf = ml_dtypes.bfloat16
    crep, srep, tri = _host_consts()
    in_maps = []
    for c in range(8):
        b, g = divmod(c, 4)
        gs = slice(g * 512, (g + 1) * 512)
        wq = Wqkv[:, 0:2048][:, gs]
        wk = Wqkv[:, 2048:4096][:, gs]
        wvv = Wqkv[:, 4096:6144][:, gs]
        bq = bqkv[0:2048][gs]
        bk = bqkv[2048:4096][gs]
        bvv = bqkv[4096:6144][gs]
        xT = np.ascontiguousarray(x[b].T)                  # [E, S]
        xcc = np.ascontiguousarray(
            xT.reshape(E, NCH, SCH).transpose(1, 0, 2))    # [NCH, E, SCH]
        wqkc = np.ascontiguousarray(
            np.concatenate([wq, wk], axis=1)               # [E, 1024]
            .reshape(E, 8, 128).transpose(1, 0, 2))        # [8, E, 128]
        in_maps.append(dict(
            xc=xcc.astype(bf),
            wqk=wqkc.astype(bf),
            wv=wvv.astype(bf),
            wout=Wout[gs, :].astype(bf),
            bqk=np.concatenate([bq, bk]).reshape(8, 128).T.astype(
                np.float32).copy(),
            bvb=np.broadcast_to(
                bvv.astype(bf), (128, 512)).copy(),
            crep=crep, srep=srep, tri=tri,
        ))
    kwargs = _CACHE.get("run_kwargs", {})
    res = run_bass_kernel_spmd(nc, in_maps, list(range(8)), **kwargs)
    _CACHE["last_results"] = res
    out = np.zeros((2, S, E), np.float32)
    for c in range(8):
        yb = np.asarray(res.results[c]["y"], np.float32)   # [16,128,E]
        out[c // 4] += yb.reshape(S, E)
    out += bout[None, None, :]
    return out


# revision 13
# speedup vs baseline: 1.3037x; 1.0399x over previous
"""Distributed MHA kernel for Trainium2 (8 NeuronCores, SPMD), v3.

Problem: b=2, s=2048, e=2048, 32 heads x 64 dim, rotary_dim=32, causal,
fp32 reference.  Sharding: core c = batch*4 + head_group, i.e. each core
handles one batch and 8 heads (tensor-parallel over heads, data-parallel
over batch).  Column-parallel Wqkv, row-parallel Wout; the 4 partial
outputs per batch are summed on the host (bf16 partials, 4 x 8.4 MB).

Per-core structure (all matmuls bf16, fp32 PSUM accumulation):
  A(tj): qkvT f-tiles for s-chunk tj; bias folded into the DVE PSUM->SBUF
         evacuation; RoPE on DVE (3 tensor_tensor ops; half-rotation via
         SBUF-SBUF DMAs on the gpsimd SWDGE queue).
  B(tj): per head-pair (f-tile) pr: scores for both heads as a row-tiled
         matmul pair (lhsT at partitions 0-63 / 64-127 -> concurrent
         32x32-subarray execution), one exp ACTIVATE over the [128,1024]
         pair tile, triangular [128,128] mask-mul on diagonal tiles only,
         pv matmuls with rhs narrowed to the un-masked column range;
         row 64 of the v-extended matmul accumulates the softmax denom.
  C(tj): output projection; 4 accumulating matmuls per (t,e) block; bf16
         row-block stores (one DMA per 128-row block).
Emission order A0 B0 A1 C0 B1 A2 B2 A3 B3 C1 C2 C3: A runs one chunk
ahead of B as tensor-engine filler, and the deferred C phases keep the
PE dense (HAM-warm) while the scalar engine works through the last
chunk's exps.  Input DMAs are batched into few large descriptors and
split across both HWDGE queues (sync + scalar) so issue serialization
does not gate the first chunk.
"""

import numpy as np

S = 2048
E = 2048
NET = 16          # e-tiles of 128
SCH = 512         # s-chunk
NCH = 4           # s-chunks


def _build_nc():
    import concourse.bacc as bacc
    import concourse.bass as bass  # noqa: F401
    import concourse.tile as tile
    from concourse import mybir

    f32 = mybir.dt.float32
    bf16 = mybir.dt.bfloat16
    AF = mybir.ActivationFunctionType

    nc = bacc.Bacc(None, target_bir_lowering=False)
    # chunk-major xT so every x load is a dense contiguous read
    xc = nc.dram_tensor("xc", [NCH, E, SCH], bf16, kind="ExternalInput")
    # f-tile-major q,k weights: wqk[o] = [E, 128] slab for f-tile o
    wqk = nc.dram_tensor("wqk", [8, E, 128], bf16, kind="ExternalInput")
    wv = nc.dram_tensor("wv", [E, 512], bf16, kind="ExternalInput")
    wout = nc.dram_tensor("wout", [512, E], bf16, kind="ExternalInput")
    bqk = nc.dram_tensor("bqk", [128, 8], f32, kind="ExternalInput")
    bvb = nc.dram_tensor("bvb", [128, 512], bf16, kind="ExternalInput")
    crep = nc.dram_tensor("crep", [128, S], bf16, kind="ExternalInput")
    srep = nc.dram_tensor("srep", [128, S], bf16, kind="ExternalInput")
    tri = nc.dram_tensor("tri", [128, 128], bf16, kind="ExternalInput")
    # block-packed output: y[tt] is one [128, E] store
    y = nc.dram_tensor("y", [16, 128, E], bf16, kind="ExternalOutput")

    with tile.TileContext(nc) as tc:
        from contextlib import ExitStack

        with ExitStack() as ctx:
            consts = ctx.enter_context(tc.tile_pool(name="consts", bufs=1))
            xp = ctx.enter_context(tc.tile_pool(name="xp", bufs=2))
            qjp = ctx.enter_context(tc.tile_pool(name="qjp", bufs=2))
            qkp = ctx.enter_context(tc.tile_pool(name="qkp", bufs=1))
            vp = ctx.enter_context(tc.tile_pool(name="vp", bufs=1))
            rtp = ctx.enter_context(tc.tile_pool(name="rtp", bufs=2))
            atp = ctx.enter_context(tc.tile_pool(name="atp", bufs=3))
            ptp = ctx.enter_context(tc.tile_pool(name="ptp", bufs=3))
            dnp = ctx.enter_context(tc.tile_pool(name="dnp", bufs=2))
            rbp = ctx.enter_context(tc.tile_pool(name="rbp", bufs=2))
            ysp = ctx.enter_context(tc.tile_pool(name="ysp", bufs=2))
            ps_a = ctx.enter_context(
                tc.tile_pool(name="ps_a", bufs=2, space="PSUM"))
            ps_s = ctx.enter_context(
                tc.tile_pool(name="ps_s", bufs=2, space="PSUM"))
            ps_o = ctx.enter_context(
                tc.tile_pool(name="ps_o", bufs=1, space="PSUM"))

            x_t = {}      # tj -> x tile [128, 16, 512]
            q_t = {}      # (pr, tj) -> q f-tile (post-rope)
            k_t = {}      # (pr, tj) -> k f-tile (post-rope)
            v_t = {}      # ut -> v tile [128, 8, 65]
            at_t = {}     # (pr, tj) -> normalized attn out (transposed)

            def load_x(tj, split=4):
                t = xp.tile([128, NET, SCH], bf16, tag="x")
                src = xc[tj].rearrange("(et p) c -> p et c", p=128)
                step = NET // split
                for g in range(split):
                    eng = nc.sync if g % 2 == 0 else nc.scalar
                    eng.dma_start(
                        t[:, g * step:(g + 1) * step, :],
                        src[:, g * step:(g + 1) * step, :])
                x_t[tj] = t

            # ---- x chunk 0 + early consts first so the PE starts ASAP;
            # loads alternate between the two HWDGE queues (sync/scalar).
            # The first chunk and first slab arrive in small pieces so the
            # first matmuls can begin within ~2us.
            load_x(0, split=8)
            w_qk = []
            for o in range(8):
                t = consts.tile([128, NET, 128], bf16, tag=f"wqk{o}")
                src = wqk[o].rearrange("(et p) c -> p et c", p=128)
                eng = nc.sync if o % 2 == 0 else nc.scalar
                if o == 0:
                    for g in range(4):
                        eng.dma_start(
                            t[:, g * 4:(g + 1) * 4, :],
                            src[:, g * 4:(g + 1) * 4, :])
                else:
                    eng.dma_start(t, src)
                w_qk.append(t)
                if o == 0:
                    crep_sb = consts.tile([128, S], bf16, tag="crep")
                    nc.scalar.dma_start(crep_sb, crep[:, :])
                    srep_sb = consts.tile([128, S], bf16, tag="srep")
                    nc.sync.dma_start(srep_sb, srep[:, :])
                    bqk_sb = consts.tile([128, 8], f32, tag="bqk")
                    nc.scalar.dma_start(bqk_sb, bqk[:, :])
            # ---- v weights: w_v = [128, 16, 512]
            w_v = consts.tile([128, NET, 512], bf16, tag="wv")
            src = wv.rearrange("(et p) c -> p et c", p=128)
            for g in range(2):
                eng = nc.sync if g % 2 == 0 else nc.scalar
                eng.dma_start(
                    w_v[:, g * 8:(g + 1) * 8, :], src[:, g * 8:(g + 1) * 8, :])
            bv_sb = consts.tile([128, 512], bf16, tag="bv")
            nc.sync.dma_start(bv_sb, bvb[:, :])
            tri_sb = consts.tile([128, 128], bf16, tag="tri")
            nc.scalar.dma_start(tri_sb, tri[:, :])
            wo_sb = consts.tile([128, 4, E], bf16, tag="wo")
            src = wout.rearrange("(pr p) c -> p pr c", p=128)
            for g in range(2):
                eng = nc.sync if g % 2 == 0 else nc.scalar
                eng.dma_start(
                    wo_sb[:, g * 2:(g + 1) * 2, :], src[:, g * 2:(g + 1) * 2, :])
            ones = consts.tile([1, 512], bf16, tag="ones")
            nc.vector.memset(ones, 1.0)

            def phase_a(tj):
                cs = slice(tj * SCH, (tj + 1) * SCH)
                xs = x_t[tj]
                for o in range(12):
                    ps = ps_a.tile([128, 512], f32, tag="a")
                    if o < 8:
                        # q (o 0-3) / k (o 4-7) f-tile: w^T x
                        for et in range(NET):
                            nc.tensor.matmul(
                                ps, lhsT=w_qk[o][:, et, :],
                                rhs=xs[:, et, :],
                                start=(et == 0), stop=(et == NET - 1))
                        pr = o if o < 4 else o - 4
                        if o < 4:
                            qt = qjp.tile([128, SCH], bf16, tag=f"q{pr}")
                            q_t[(pr, tj)] = qt
                        else:
                            qt = qkp.tile([128, SCH], bf16, tag=f"k{pr}_{tj}")
                            k_t[(pr, tj)] = qt
                        # PSUM->SBUF evacuation with per-partition bias
                        nc.vector.tensor_scalar_add(
                            qt, ps, bqk_sb[:, o:o + 1])
                        # RoPE: tmp = within-32-block 16-row swap of qt
                        # (rows 32-63/96-127 copied straight; srep is 0
                        # there).  SBUF-SBUF DMAs ride the gpsimd SWDGE
                        # queue to keep the HWDGE queues for loads.
                        tmp = rtp.tile([128, SCH], bf16, tag="rtmp")
                        for hh in (0, 64):
                            nc.gpsimd.dma_start(
                                tmp[hh:hh + 16, :], qt[hh + 16:hh + 32, :])
                            nc.gpsimd.dma_start(
                                tmp[hh + 16:hh + 32, :], qt[hh:hh + 16, :])
                            nc.gpsimd.dma_start(
                                tmp[hh + 32:hh + 64, :], qt[hh + 32:hh + 64, :])
                        nc.vector.tensor_mul(qt, qt, crep_sb[:, cs])
                        nc.vector.tensor_mul(tmp, tmp, srep_sb[:, cs])
                        nc.vector.tensor_add(qt, qt, tmp)
                    else:
                        us = o - 8
                        ut = tj * 4 + us
                        for et in range(NET):
                            nc.tensor.matmul(
                                ps,
                                lhsT=xs[:, et, us * 128:(us + 1) * 128],
                                rhs=w_v[:, et, :],
                                start=(et == 0), stop=(et == NET - 1))
                        vt = vp.tile([128, 8, 65], bf16, tag=f"v{ut}")
                        nc.vector.tensor_add(
                            vt[:, :, 0:64],
                            ps.rearrange("p (h d) -> p h d", h=8),
                            bv_sb.rearrange("p (h d) -> p h d", h=8))
                        nc.vector.memset(vt[:, :, 64:65], 1.0)
                        v_t[ut] = vt

            def phase_b(tj, filler=None):
                nu = 4 * tj + 4
                for pr in range(4):
                    h0, h1 = 2 * pr, 2 * pr + 1
                    oTa = ps_o.tile([65, 512], f32, tag="o0")
                    oTb = ps_o.tile([65, 512], f32, tag="o1")
                    qt = q_t[(pr, tj)]
                    for ut in range(nu):
                        jj, us = divmod(ut, 4)
                        kk = ut - 4 * tj  # >=0: diagonal tile index
                        kt = k_t[(pr, jj)]
                        pp = ps_s.tile([128, 1024], f32, tag="s")
                        # scores for both heads as a concurrent row-tiled
                        # pair (lhsT partitions 0-63 / 64-127)
                        nc.tensor.matmul(
                            pp[:, 0:512],
                            lhsT=kt[0:64, us * 128:(us + 1) * 128],
                            rhs=qt[0:64, :], start=True, stop=True)
                        nc.tensor.matmul(
                            pp[:, 512:1024],
                            lhsT=kt[64:128, us * 128:(us + 1) * 128],
                            rhs=qt[64:128, :], start=True, stop=True)
                        pt = ptp.tile([128, 1024], bf16, tag="pt")
                        nc.scalar.activation(pt, pp, AF.Exp, scale=0.125)
                        off = 0
                        if kk >= 0:
                            off = 128 * kk
                            nc.vector.tensor_mul(
                                pt[:, off:off + 128],
                                pt[:, off:off + 128], tri_sb)
                            nc.vector.tensor_mul(
                                pt[:, 512 + off:512 + off + 128],
                                pt[:, 512 + off:512 + off + 128], tri_sb)
                        nc.tensor.matmul(
                            oTa[:, off:512], lhsT=v_t[ut][:, h0, :],
                            rhs=pt[:, off:512],
                            start=(ut == 0), stop=(ut == nu - 1))
                        nc.tensor.matmul(
                            oTb[:, off:512], lhsT=v_t[ut][:, h1, :],
                            rhs=pt[:, 512 + off:1024],
                            start=(ut == 0), stop=(ut == nu - 1))
                        # interleave deferred C work so the PE program has
                        # ready filler while the scalar engine chews exps
                        if filler is not None and ut % 8 == 7:
                            next(filler, None)
                    # denominators -> reciprocal -> broadcast via K=1 matmul
                    # (denominator rows go PSUM->SBUF first: the custom-DVE
                    # recip uop chain is not reliable with a PSUM source)
                    dna = dnp.tile([1, 512], f32, tag="dn0")
                    nc.vector.tensor_copy(dna, oTa[64:65, :])
                    dnb = dnp.tile([1, 512], f32, tag="dn1")
                    nc.vector.tensor_copy(dnb, oTb[64:65, :])
                    rca = dnp.tile([1, 512], f32, tag="rc0")
                    nc.vector.reciprocal_approx_fast(out=rca, in_=dna)
                    rcb = dnp.tile([1, 512], f32, tag="rc1")
                    nc.vector.reciprocal_approx_fast(out=rcb, in_=dnb)
                    rba = dnp.tile([1, 512], bf16, tag="rcb0")
                    nc.vector.tensor_copy(rba, rca)
                    rbb = dnp.tile([1, 512], bf16, tag="rcb1")
                    nc.vector.tensor_copy(rbb, rcb)
                    rb_ps = ps_s.tile([128, 512], f32, tag="s")
                    nc.tensor.matmul(rb_ps[0:64, :], lhsT=ones[:, 0:64],
                                     rhs=rba, start=True, stop=True)
                    nc.tensor.matmul(rb_ps[64:128, :], lhsT=ones[:, 0:64],
                                     rhs=rbb, start=True, stop=True)
                    rb_sb = rbp.tile([128, 512], bf16, tag="rb")
                    nc.vector.tensor_copy(rb_sb, rb_ps)
                    at = atp.tile([128, 512], bf16, tag=f"at{pr}")
                    nc.vector.tensor_mul(
                        at[0:64, :], oTa[0:64, :], rb_sb[0:64, :])
                    nc.vector.tensor_mul(
                        at[64:128, :], oTb[0:64, :], rb_sb[64:128, :])
                    at_t[(pr, tj)] = at

            def c_block(tj, ttl):
                tt = tj * 4 + ttl
                ys = ysp.tile([128, E], bf16, tag="ys")
                for ec in range(4):
                    yp = ps_s.tile([128, 512], f32, tag="s")
                    for pr in range(4):
                        nc.tensor.matmul(
                            yp,
                            lhsT=at_t[(pr, tj)][:, ttl * 128:(ttl + 1) * 128],
                            rhs=wo_sb[:, pr, ec * 512:(ec + 1) * 512],
                            start=(pr == 0), stop=(pr == 3))
                    # evacuate on the scalar engine: it is idle in C-heavy
                    # stretches while the DVE carries the denominator work
                    nc.scalar.activation(
                        ys[:, ec * 512:(ec + 1) * 512], yp, AF.Copy)
                nc.sync.dma_start(y[tt, :, :], ys)

            def phase_c(tj):
                for ttl in range(4):
                    c_block(tj, ttl)

            def c_filler(tjs):
                for tj in tjs:
                    for ttl in range(4):
                        yield c_block(tj, ttl)

            # emission order == scheduling priority: B (which feeds the
            # scalar engine) hot, A one chunk ahead as PE filler, C(1..2)
            # interleaved into B(3)'s emission so the in-order PE program
            # has ready work while the scalar engine chews the tail exps.
            phase_a(0)
            load_x(1)
            phase_b(0)
            phase_a(1)
            phase_c(0)
            load_x(2)
            phase_b(1)
            phase_a(2)
            load_x(3)
            phase_b(2)
            phase_a(3)
            fill = c_filler((1, 2))
            phase_b(3, filler=fill)
            for _ in fill:
                pass
            phase_c(3)
    nc.compile()
    return nc


_CACHE = {}


def _host_consts():
    import ml_dtypes
    bf = ml_dtypes.bfloat16
    inv = 1.0 / (10000.0 ** (np.arange(0, 32, 2, dtype=np.float64) / 32.0))
    t = np.arange(S, dtype=np.float64)
    fr = np.outer(t, inv)                       # [s, 16]
    cos = np.cos(fr).astype(np.float32).T       # [16, s]
    sin = np.sin(fr).astype(np.float32).T
    crep = np.ones((128, S), np.float32)
    srep = np.zeros((128, S), np.float32)
    for blk in (0, 64):
        crep[blk:blk + 16] = cos
        crep[blk + 16:blk + 32] = cos
        srep[blk:blk + 16] = -sin
        srep[blk + 16:blk + 32] = sin
    ui = np.arange(128)[:, None]
    cc = np.arange(128)[None, :]
    tri = (ui <= cc).astype(np.float32)         # keep[u, c]
    return crep.astype(bf), srep.astype(bf), tri.astype(bf)


def kernel(**inputs):
    import ml_dtypes
    from concourse.bass_utils import run_bass_kernel_spmd

    x = np.asarray(inputs["x"], np.float32)
    Wqkv = np.asarray(inputs["Wqkv"], np.float32)
    bqkv = np.asarray(inputs["bqkv"], np.float32)
    Wout = np.asarray(inputs["Wout"], np.float32)
    bout = np.asarray(inputs["bout"], np.float32)

    if "nc" not in _CACHE:
        _CACHE["nc"] = _build_nc()
    nc = _CACHE["nc"]

    bf = ml_dtypes.bfloat16
    crep, srep, tri = _host_consts()
    in_maps = []
    for c in range(8):
        b, g = divmod(c, 4)
        gs = slice(g * 512, (g + 1) * 512)
        wq = Wqkv[:, 0:2048][:, gs]
        wk = Wqkv[:, 2048:4096][:, gs]
        wvv = Wqkv[:, 4096:6144][:, gs]
        bq = bqkv[0:2048][gs]
        bk = bqkv[2048:4096][gs]
        bvv = bqkv[4096:6144][gs]
        xT = np.ascontiguousarray(x[b].T)                  # [E, S]
        xcc = np.ascontiguousarray(
            xT.reshape(E, NCH, SCH).transpose(1, 0, 2))    # [NCH, E, SCH]
        wqkc = np.ascontiguousarray(
            np.concatenate([wq, wk], axis=1)               # [E, 1024]
            .reshape(E, 8, 128).transpose(1, 0, 2))        # [8, E, 128]
        in_maps.append(dict(
            xc=xcc.astype(bf),
            wqk=wqkc.astype(bf),
            wv=wvv.astype(bf),
            wout=Wout[gs, :].astype(bf),
            bqk=np.concatenate([bq, bk]).reshape(8, 128).T.astype(
                np.float32).copy(),
            bvb=np.broadcast_to(
                bvv.astype(bf), (128, 512)).copy(),
            crep=crep, srep=srep, tri=tri,
        ))
    kwargs = _CACHE.get("run_kwargs", {})
    res = run_bass_kernel_spmd(nc, in_maps, list(range(8)), **kwargs)
    _CACHE["last_results"] = res
    out = np.zeros((2, S, E), np.float32)
    for c in range(8):
        yb = np.asarray(res.results[c]["y"], np.float32)   # [16,128,E]
        out[c // 4] += yb.reshape(S, E)
    out += bout[None, None, :]
    return out
